# revision 1
# baseline (speedup 1.0000x reference)
"""ConvLSTM2D cell on 8 Trainium2 NeuronCores.

Data-parallel over batch: B=16 -> 2 images per core.

fp8 path (default): the 3x3 conv over concat([x, h]) is computed with
fp8e4m3 DoubleRow matmuls (2 K=128 tiles contracted per instruction at
0.5 cycles/output-column -- 2x the fp16 rate). Precision is recovered
with a 3-term residual expansion: w*a ~= w8*a8 + w8*ar + wr*a8, where
w8/a8 are fp8 quantizations (weights pre-scaled x32 into fp8 normal
range) and wr/ar are fp8 quantizations of the leftover residuals stored
at the SAME scale (they land in fp8's subnormal range, which is exactly
fine: the absolute subnormal step 2^-9 bounds the residual error).
All three terms share one PSUM accumulator; the x32 weight scale is
undone by the activation's scale parameter. Measured end-to-end error
~5e-3 vs the f32 reference (vs ~1e-3 for the fp16 path).

Per (gate, 512-pixel chunk): 21 DoubleRow matmuls (7 per term) instead
of 14 fp16 matmuls. Each DR matmul contracts two conv taps' K-tiles
selected by a strided access pattern over one padded SBUF image tile
img[128ch, 3 planes, 66, 66] (plane 0 = padded h; plane 1 = x packed
with x-shifted-left in channels 64-127; plane 2 = x packed with
x-shifted-up), so every tap pair is a pure AP offset. ScalarE applies
scale+bias+sigmoid/tanh straight out of PSUM; VectorE does the LSTM
elementwise math.

fp16 path kept for fallback/AB (dt_mm=float16): 14 matmuls per (gate,
chunk), see git history for details.
"""

import sys

if "/opt/trn_rl_repo" not in sys.path:
    sys.path.insert(0, "/opt/trn_rl_repo")

import numpy as np

import concourse.bass as bass
import concourse.tile as tile
from bass_rust import AP
from concourse import bacc, mybir
from concourse.bass_utils import run_bass_kernel_spmd

N_CORES = 8
B, C_IN, C_HID, H, W = 16, 64, 128, 64, 64
B_LOC = B // N_CORES  # 2 images per core
HP = H + 2  # padded
WP = W + 2
PL = HP * WP  # plane size in elements
ROWS_PER_CHUNK = 8  # 8 rows x 64 cols = 512 pixels per matmul chunk
NCH = H // ROWS_PER_CHUNK  # chunks per image
TAPS = [(kh, kw) for kh in range(3) for kw in range(3)]
W_SCALE = 32.0  # fp8 weight pre-scale (undone in the activation)

# fp8 image layout: 5 pre-shifted 64-wide planes per image, each 66 rows
# (1 zero guard row top+bottom). Horizontal tap shifts are baked into the
# planes so every 8x64 window is a single CONTIGUOUS 512-element block --
# DoubleRow rhs APs stay 3-dim [128, 2, 512] (4-dim strided DR ifmaps
# crash the device). Planes: 0..2 = h shifted dw=-1/0/+1; 3 = xp pack
# [x(dw=-1) | x(dw=0)] in channel halves; 4 = xq pack [x(dw=+1) |
# x(dw=+1) shifted up one row].
N_PLANES = 5
PS = (H + 2) * W  # plane size: 66 rows x 64 cols

# DoubleRow pair table: (plane of ktile0, dv of ktile0, ktile delta).
# dv is the vertical tap shift (kh-1); ktile1 sits at +delta.
_DR_PAIRS = [
    (0, -1, PS),  # h(0,0) + h(0,1)
    (0, 0, PS),  # h(1,0) + h(1,1)
    (0, 1, PS),  # h(2,0) + h(2,1)
    (2, -1, W),  # h(0,2) + h(1,2)
    (3, -1, W),  # x(0,0)+x(0,1) + x(1,0)+x(1,1)
    (3, 1, PS - 2 * W),  # x(2,0)+x(2,1) + x(0,2)+x(1,2)
    (2, 1, 2 * PS),  # h(2,2) + x(2,2) (ktile1 ch64-127 weights are 0)
]

_cache = {}


def _dr_view(t, p, h0):
    """rhs AP [128, 2, 512] for DoubleRow pair p at chunk row h0 on an
    image tile t of shape [128, N_PLANES, 66, 64]."""
    plane, dv, delta = _DR_PAIRS[p]
    full = t[:]
    off = full.offset + plane * PS + (h0 + dv + 1) * W
    return AP(full.tensor, off, [full.ap[0], (delta, 2), (1, ROWS_PER_CHUNK * W)])


def _build_fp8(trace=False, unroll=1):
    f32 = mybir.dt.float32
    f8 = mybir.dt.float8e4
    nc = bacc.Bacc("TRN2", target_bir_lowering=False, debug=False, num_devices=N_CORES)

    img8_ap = nc.dram_tensor("img8", [B_LOC, 128, N_PLANES, H + 2, W], f8, kind="ExternalInput").ap()
    imgr8_ap = nc.dram_tensor("imgr8", [B_LOC, 128, N_PLANES, H + 2, W], f8, kind="ExternalInput").ap()
    c_ap = nc.dram_tensor("c", [B_LOC, C_HID, H * W], f32, kind="ExternalInput").ap()
    # wdr: [K=128, set(0=w8,1=wr), gate, pair, ktile, M=128]
    wdr_ap = nc.dram_tensor("wdr", [128, 2, 4, 7, 2, 128], f8, kind="ExternalInput").ap()
    bias_ap = nc.dram_tensor("biasT", [C_HID, 4], f32, kind="ExternalInput").ap()
    hn_ap = nc.dram_tensor("hn", [B_LOC, C_HID, H * W], f32, kind="ExternalOutput").ap()
    cn_ap = nc.dram_tensor("cn", [B_LOC, C_HID, H * W], f32, kind="ExternalOutput").ap()

    with tile.TileContext(nc) as tc:
        with (
            tc.tile_pool(name="weights", bufs=1) as wpool,
            tc.tile_pool(name="imgs", bufs=2) as ipool,
            tc.tile_pool(name="cstate", bufs=3) as cpool,
            tc.tile_pool(name="psum", bufs=8, space="PSUM") as ppool,
            tc.tile_pool(name="acts", bufs=2) as apool,
            tc.tile_pool(name="outs", bufs=3) as opool,
        ):
            w_t = wpool.tile([128, 2, 4, 7, 2, 128], f8, tag="wdr")
            bias_t = wpool.tile([C_HID, 4], f32, tag="bias")
            nc.sync.dma_start(w_t[:], wdr_ap[:])
            nc.sync.dma_start(bias_t[:], bias_ap[:])

            i8 = []
            ir8 = []
            for b in range(B_LOC):
                i8_b = ipool.tile([128, N_PLANES, H + 2, W], f8, tag="img8")
                ir8_b = ipool.tile([128, N_PLANES, H + 2, W], f8, tag="imgr8")
                nc.sync.dma_start(i8_b[:], img8_ap[b])
                nc.sync.dma_start(ir8_b[:], imgr8_ap[b])
                i8.append(i8_b)
                ir8.append(ir8_b)

            dr = mybir.MatmulPerfMode.DoubleRow
            for _rep in range(unroll):
                for b in range(B_LOC):
                    for ch in range(NCH):
                        h0 = ch * ROWS_PER_CHUNK
                        c_sl = cpool.tile([C_HID, 512], f32, tag="c", name=f"c_{_rep}_{b}_{ch}")
                        nc.sync.dma_start(c_sl[:], c_ap[b][:, h0 * W : (h0 + ROWS_PER_CHUNK) * W])
                        gate_sb = []
                        for g in range(4):
                            acc = ppool.tile([C_HID, 512], f32, tag="acc")
                            for p in range(7):
                                v8 = _dr_view(i8[b], p, h0)
                                vr = _dr_view(ir8[b], p, h0)
                                # w8*a8, w8*ar (same stationary), wr*a8
                                nc.tensor.matmul(
                                    acc[:], w_t[:, 0, g, p], v8,
                                    start=(p == 0), stop=False, perf_mode=dr,
                                )
                                nc.tensor.matmul(
                                    acc[:], w_t[:, 0, g, p], vr,
                                    start=False, stop=False, perf_mode=dr,
                                )
                                nc.tensor.matmul(
                                    acc[:], w_t[:, 1, g, p], v8,
                                    start=False, stop=(p == 6), perf_mode=dr,
                                )
                            act_t = apool.tile([C_HID, 512], f32, tag=f"gate{g}")
                            func = (
                                mybir.ActivationFunctionType.Tanh
                                if g == 3
                                else mybir.ActivationFunctionType.Sigmoid
                            )
                            nc.scalar.activation(
                                act_t[:], acc[:], func,
                                bias=bias_t[:, g : g + 1], scale=1.0 / W_SCALE,
                            )
                            gate_sb.append(act_t)

                        i_t, f_t, o_t, g_t = gate_sb
                        ig = opool.tile([C_HID, 512], f32, tag="ig")
                        nc.vector.tensor_mul(ig[:], i_t[:], g_t[:])
                        fc = opool.tile([C_HID, 512], f32, tag="fc")
                        nc.vector.tensor_mul(fc[:], f_t[:], c_sl[:])
                        cn_t = opool.tile([C_HID, 512], f32, tag="cn")
                        nc.vector.tensor_add(cn_t[:], fc[:], ig[:])
                        nc.sync.dma_start(cn_ap[b][:, h0 * W : (h0 + ROWS_PER_CHUNK) * W], cn_t[:])
                        th_t = opool.tile([C_HID, 512], f32, tag="th")
                        nc.scalar.activation(th_t[:], cn_t[:], mybir.ActivationFunctionType.Tanh)
                        hn_t = opool.tile([C_HID, 512], f32, tag="hn")
                        nc.vector.tensor_mul(hn_t[:], o_t[:], th_t[:])
                        nc.sync.dma_start(hn_ap[b][:, h0 * W : (h0 + ROWS_PER_CHUNK) * W], hn_t[:])

    nc.compile()
    return nc


def _build_fp16(dt_mm, trace=False, unroll=1):
    f32 = mybir.dt.float32
    nc = bacc.Bacc("TRN2", target_bir_lowering=False, debug=False, num_devices=N_CORES)

    x_ap = nc.dram_tensor("x", [B_LOC, 2 * C_IN, HP, WP], dt_mm, kind="ExternalInput").ap()
    x2_ap = nc.dram_tensor("x2", [B_LOC, 2 * C_IN, HP, WP], dt_mm, kind="ExternalInput").ap()
    h_ap = nc.dram_tensor("h", [B_LOC, C_HID, HP, WP], dt_mm, kind="ExternalInput").ap()
    c_ap = nc.dram_tensor("c", [B_LOC, C_HID, H * W], f32, kind="ExternalInput").ap()
    wxp_ap = nc.dram_tensor("wxp", [3, 2 * C_IN, 4 * C_HID], dt_mm, kind="ExternalInput").ap()
    wxr_ap = nc.dram_tensor("wxr", [2 * C_IN, 4 * C_HID], dt_mm, kind="ExternalInput").ap()
    wx3_ap = nc.dram_tensor("wx3", [C_IN, 4 * C_HID], dt_mm, kind="ExternalInput").ap()
    wh_ap = nc.dram_tensor("wh", [9, C_HID, 4 * C_HID], dt_mm, kind="ExternalInput").ap()
    bias_ap = nc.dram_tensor("biasT", [C_HID, 4], f32, kind="ExternalInput").ap()
    hn_ap = nc.dram_tensor("hn", [B_LOC, C_HID, H * W], f32, kind="ExternalOutput").ap()
    cn_ap = nc.dram_tensor("cn", [B_LOC, C_HID, H * W], f32, kind="ExternalOutput").ap()

    with tile.TileContext(nc) as tc:
        with (
            tc.tile_pool(name="weights", bufs=1) as wpool,
            tc.tile_pool(name="imgs", bufs=2) as ipool,
            tc.tile_pool(name="cstate", bufs=3) as cpool,
            tc.tile_pool(name="psum", bufs=8, space="PSUM") as ppool,
            tc.tile_pool(name="acts", bufs=2) as apool,
            tc.tile_pool(name="outs", bufs=3) as opool,
        ):
            wh_t = wpool.tile([C_HID, 9, 4 * C_HID], dt_mm, tag="wh")
            wxp_t = wpool.tile([2 * C_IN, 3, 4 * C_HID], dt_mm, tag="wxp")
            wxr_t = wpool.tile([2 * C_IN, 4 * C_HID], dt_mm, tag="wxr")
            wx3_t = wpool.tile([C_IN, 4 * C_HID], dt_mm, tag="wx3")
            bias_t = wpool.tile([C_HID, 4], f32, tag="bias")
            nc.sync.dma_start(wh_t[:], wh_ap.rearrange("t k m -> k t m"))
            nc.sync.dma_start(wxp_t[:], wxp_ap.rearrange("t k m -> k t m"))
            nc.sync.dma_start(wxr_t[:], wxr_ap[:])
            nc.sync.dma_start(wx3_t[:], wx3_ap[:])
            nc.sync.dma_start(bias_t[:], bias_ap[:])

            hp = []
            xp = []
            xq = []
            for b in range(B_LOC):
                hp_b = ipool.tile([C_HID, HP, WP], dt_mm, tag="hp")
                xp_b = ipool.tile([2 * C_IN, HP, WP], dt_mm, tag="xp")
                xq_b = ipool.tile([2 * C_IN, HP, WP], dt_mm, tag="xq")
                nc.sync.dma_start(hp_b[:], h_ap[b])
                nc.sync.dma_start(xp_b[:], x_ap[b])
                nc.sync.dma_start(xq_b[:], x2_ap[b])
                hp.append(hp_b)
                xp.append(xp_b)
                xq.append(xq_b)

            for _rep in range(unroll):
                for b in range(B_LOC):
                    for ch in range(NCH):
                        h0 = ch * ROWS_PER_CHUNK
                        c_sl = cpool.tile([C_HID, 512], f32, tag="c", name=f"c_{_rep}_{b}_{ch}")
                        nc.sync.dma_start(c_sl[:], c_ap[b][:, h0 * W : (h0 + ROWS_PER_CHUNK) * W])
                        gate_sb = []
                        for g in range(4):
                            acc = ppool.tile([C_HID, 512], f32, tag="acc")
                            n_mm = 14
                            mm = 0
                            for kh, kw in TAPS:
                                nc.tensor.matmul(
                                    acc[:],
                                    wh_t[:, kh * 3 + kw, g * C_HID : (g + 1) * C_HID],
                                    hp[b][:, h0 + kh : h0 + kh + ROWS_PER_CHUNK, kw : kw + W],
                                    start=(mm == 0),
                                    stop=(mm == n_mm - 1),
                                )
                                mm += 1
                            for kh in range(3):
                                nc.tensor.matmul(
                                    acc[:],
                                    wxp_t[:, kh, g * C_HID : (g + 1) * C_HID],
                                    xp[b][:, h0 + kh : h0 + kh + ROWS_PER_CHUNK, 0:W],
                                    start=(mm == 0),
                                    stop=(mm == n_mm - 1),
                                )
                                mm += 1
                            nc.tensor.matmul(
                                acc[:],
                                wxr_t[:, g * C_HID : (g + 1) * C_HID],
                                xq[b][:, h0 : h0 + ROWS_PER_CHUNK, 2 : 2 + W],
                                start=(mm == 0),
                                stop=(mm == n_mm - 1),
                            )
                            mm += 1
                            nc.tensor.matmul(
                                acc[:],
                                wx3_t[:, g * C_HID : (g + 1) * C_HID],
                                xp[b][0:C_IN, h0 + 2 : h0 + 2 + ROWS_PER_CHUNK, 2 : 2 + W],
                                start=(mm == 0),
                                stop=(mm == n_mm - 1),
                            )
                            mm += 1
                            act_t = apool.tile([C_HID, 512], f32, tag=f"gate{g}")
                            func = (
                                mybir.ActivationFunctionType.Tanh
                                if g == 3
                                else mybir.ActivationFunctionType.Sigmoid
                            )
                            nc.scalar.activation(act_t[:], acc[:], func, bias=bias_t[:, g : g + 1])
                            gate_sb.append(act_t)

                        i_t, f_t, o_t, g_t = gate_sb
                        ig = opool.tile([C_HID, 512], f32, tag="ig")
                        nc.vector.tensor_mul(ig[:], i_t[:], g_t[:])
                        fc = opool.tile([C_HID, 512], f32, tag="fc")
                        nc.vector.tensor_mul(fc[:], f_t[:], c_sl[:])
                        cn_t = opool.tile([C_HID, 512], f32, tag="cn")
                        nc.vector.tensor_add(cn_t[:], fc[:], ig[:])
                        nc.sync.dma_start(cn_ap[b][:, h0 * W : (h0 + ROWS_PER_CHUNK) * W], cn_t[:])
                        th_t = opool.tile([C_HID, 512], f32, tag="th")
                        nc.scalar.activation(th_t[:], cn_t[:], mybir.ActivationFunctionType.Tanh)
                        hn_t = opool.tile([C_HID, 512], f32, tag="hn")
                        nc.vector.tensor_mul(hn_t[:], o_t[:], th_t[:])
                        nc.sync.dma_start(hn_ap[b][:, h0 * W : (h0 + ROWS_PER_CHUNK) * W], hn_t[:])

    nc.compile()
    return nc


def _build(dt_mm=mybir.dt.float8e4, trace=False, unroll=1):
    key = (dt_mm, trace, unroll)
    if key in _cache:
        return _cache[key]
    if dt_mm == mybir.dt.float8e4:
        nc = _build_fp8(trace, unroll)
    else:
        nc = _build_fp16(dt_mm, trace, unroll)
    _cache[key] = nc
    return nc


def _pad_images(x, h_cur):
    """Returns [B, 128, 3, HP, WP] f32: plane 0 = padded h, plane 1 = xp
    (x | x-left), plane 2 = xq (x | x-up)."""
    hpad = np.zeros((B, C_HID, HP, WP), dtype=np.float32)
    hpad[:, :, 1 : H + 1, 1 : W + 1] = h_cur
    xpad = np.zeros((B, 2 * C_IN, HP, WP), dtype=np.float32)
    xpad[:, :C_IN, 1 : H + 1, 1 : W + 1] = x
    xpad[:, C_IN:, :, : WP - 1] = xpad[:, :C_IN, :, 1:]
    xqpad = np.zeros((B, 2 * C_IN, HP, WP), dtype=np.float32)
    xqpad[:, :C_IN] = xpad[:, :C_IN]
    xqpad[:, C_IN:, : HP - 1, :] = xpad[:, :C_IN, 1:, :]
    return np.stack([hpad, xpad, xqpad], axis=2)


def _shift_w(a, dw):
    """Horizontal shift with zero fill: out[..., c] = a[..., c + dw]."""
    out = np.zeros_like(a)
    if dw == 0:
        out[:] = a
    elif dw == 1:
        out[..., : W - 1] = a[..., 1:]
    else:
        out[..., 1:] = a[..., : W - 1]
    return out


def _fp8_planes(x, h_cur):
    """[B, 128, N_PLANES, 66, 64] f32. Planes 0-2: h at dw=-1/0/+1 with one
    zero guard row top+bottom. Plane 3: [x(dw=-1) | x(dw=0)] channel halves.
    Plane 4: [x(dw=+1) | x(dw=+1) up one row] (ch 64-127 laid out so the
    dv=-1 window reads x(dw=+1) at dv=0)."""
    planes = np.zeros((B, 128, N_PLANES, H + 2, W), dtype=np.float32)
    for i, dw in enumerate([-1, 0, 1]):
        planes[:, :, i, 1 : H + 1, :] = _shift_w(h_cur, dw)
    xp1 = _shift_w(x, 1)
    planes[:, :C_IN, 3, 1 : H + 1, :] = _shift_w(x, -1)
    planes[:, C_IN:, 3, 1 : H + 1, :] = x
    planes[:, :C_IN, 4, 1 : H + 1, :] = xp1
    planes[:, C_IN:, 4, 0:H, :] = xp1
    return planes


def _prep_inputs_fp8(x, h_cur, c_cur, weight, bias):
    np8 = mybir.dt.np(mybir.dt.float8e4)
    img = _fp8_planes(x, h_cur)  # [B, 128, N_PLANES, 66, 64] f32
    img8 = img.astype(np8)
    imgr8 = (img - img8.astype(np.float32)).astype(np8)

    # weights: [512, 192, 3, 3] -> [3, 3, 192, 512], pre-scale x32, split
    wt = np.ascontiguousarray(weight.transpose(2, 3, 1, 0)).astype(np.float32) * W_SCALE
    w8 = wt.astype(np8)
    wr = (wt - w8.astype(np.float32)).astype(np8)

    wdr = np.zeros((128, 2, 4, 7, 2, 128), dtype=np8)
    for s, S in enumerate([w8, wr]):
        Sf = S  # fp8 array; slicing fine
        Sh = Sf[:, :, C_IN:, :]  # [3,3,128,512]
        Sx = Sf[:, :, :C_IN, :]  # [3,3,64,512]
        for g in range(4):
            msl = slice(g * C_HID, (g + 1) * C_HID)
            pairs = [
                (Sh[0, 0, :, msl], Sh[0, 1, :, msl]),
                (Sh[1, 0, :, msl], Sh[1, 1, :, msl]),
                (Sh[2, 0, :, msl], Sh[2, 1, :, msl]),
                (Sh[0, 2, :, msl], Sh[1, 2, :, msl]),
                (
                    np.concatenate([Sx[0, 0, :, msl], Sx[0, 1, :, msl]], axis=0),
                    np.concatenate([Sx[1, 0, :, msl], Sx[1, 1, :, msl]], axis=0),
                ),
                (
                    np.concatenate([Sx[2, 0, :, msl], Sx[2, 1, :, msl]], axis=0),
                    np.concatenate([Sx[0, 2, :, msl], Sx[1, 2, :, msl]], axis=0),
                ),
                (
                    Sh[2, 2, :, msl],
                    np.concatenate(
                        [Sx[2, 2, :, msl], np.zeros((C_IN, C_HID), dtype=np8)], axis=0
                    ),
                ),
            ]
            for p, (kt0, kt1) in enumerate(pairs):
                wdr[:, s, g, p, 0, :] = kt0
                wdr[:, s, g, p, 1, :] = kt1

    biasT = np.ascontiguousarray(bias.reshape(4, C_HID).T, dtype=np.float32)
    c3 = np.ascontiguousarray(c_cur.reshape(B, C_HID, H * W), dtype=np.float32)

    in_maps = []
    for i in range(N_CORES):
        s = slice(i * B_LOC, (i + 1) * B_LOC)
        in_maps.append(
            {
                "img8": np.ascontiguousarray(img8[s]),
                "imgr8": np.ascontiguousarray(imgr8[s]),
                "c": c3[s],
                "wdr": wdr,
                "biasT": biasT,
            }
        )
    return in_maps


def _prep_inputs_fp16(x, h_cur, c_cur, weight, bias, dt_mm):
    if dt_mm == mybir.dt.bfloat16:
        import ml_dtypes

        cast = lambda a: np.asarray(a, dtype=ml_dtypes.bfloat16)
    elif dt_mm == mybir.dt.float16:
        cast = lambda a: np.asarray(a, dtype=np.float16)
    else:
        cast = lambda a: np.ascontiguousarray(a, dtype=np.float32)

    wt = np.ascontiguousarray(weight.transpose(2, 3, 1, 0))  # [3,3,ci,co]
    wx = wt[:, :, :C_IN, :]
    wh = cast(wt[:, :, C_IN:, :].reshape(9, C_HID, 4 * C_HID))
    wxp = cast(np.concatenate([wx[:, 0, :, :], wx[:, 1, :, :]], axis=1))
    wxr = cast(np.concatenate([wx[0, 2, :, :], wx[1, 2, :, :]], axis=0))
    wx3 = cast(np.ascontiguousarray(wx[2, 2, :, :]))
    biasT = np.ascontiguousarray(bias.reshape(4, C_HID).T, dtype=np.float32)
    c3 = np.ascontiguousarray(c_cur.reshape(B, C_HID, H * W), dtype=np.float32)

    img = _pad_images(x, h_cur)  # [B, 128, 3, HP, WP]
    hpad = img[:, :, 0]
    xpad = img[:, :, 1]
    x2pad = img[:, :, 2]

    in_maps = []
    for i in range(N_CORES):
        s = slice(i * B_LOC, (i + 1) * B_LOC)
        in_maps.append(
            {
                "x": cast(xpad[s]),
                "x2": cast(x2pad[s]),
                "h": cast(hpad[s]),
                "c": c3[s],
                "wxp": wxp,
                "wxr": wxr,
                "wx3": wx3,
                "wh": wh,
                "biasT": biasT,
            }
        )
    return in_maps


def _prep_inputs(x, h_cur, c_cur, weight, bias, dt_mm=mybir.dt.float8e4):
    if dt_mm == mybir.dt.float8e4:
        return _prep_inputs_fp8(x, h_cur, c_cur, weight, bias)
    return _prep_inputs_fp16(x, h_cur, c_cur, weight, bias, dt_mm)


def run(x, h_cur, c_cur, weight, bias, dt_mm=mybir.dt.float8e4, trace=False):
    x = np.asarray(x)
    h_cur = np.asarray(h_cur)
    c_cur = np.asarray(c_cur)
    weight = np.asarray(weight)
    bias = np.asarray(bias)
    nc = _build(dt_mm, trace)
    in_maps = _prep_inputs(x, h_cur, c_cur, weight, bias, dt_mm)
    res = run_bass_kernel_spmd(nc, in_maps, list(range(N_CORES)), trace=trace)
    hn = np.concatenate([res.results[i]["hn"] for i in range(N_CORES)], axis=0)
    cn = np.concatenate([res.results[i]["cn"] for i in range(N_CORES)], axis=0)
    hn = hn.reshape(B, C_HID, H, W).astype(np.float32)
    cn = cn.reshape(B, C_HID, H, W).astype(np.float32)
    return (hn, cn), res


def kernel(x, h_cur, c_cur, weight, bias):
    (hn, cn), _ = run(x, h_cur, c_cur, weight, bias)
    return hn, cn


def _make_timing_fn(nc, in_maps):
    """Non-donating jitted runner with device-resident inputs, for
    throughput timing (slope of wall time vs iteration count)."""
    import jax
    from jax.sharding import NamedSharding

    from concourse import bass2jax, mybir as _mybir

    bass2jax.install_neuronx_cc_hook()
    n_cores = len(in_maps)
    partition_name = nc.partition_id_tensor.name if nc.partition_id_tensor else None
    in_names, out_names, out_avals, zero_outs = [], [], [], []
    for alloc in nc.m.functions[0].allocations:
        if not isinstance(alloc, _mybir.MemoryLocationSet):
            continue
        name = alloc.memorylocations[0].name
        if alloc.kind == "ExternalInput":
            if name != partition_name:
                in_names.append(name)
        elif alloc.kind == "ExternalOutput":
            out_names.append(name)
            shape = tuple(alloc.tensor_shape)
            dtype = _mybir.dt.np(alloc.dtype)
            out_avals.append(jax.core.ShapedArray(shape, dtype))
            zero_outs.append(np.zeros(shape, dtype))
    n_params = len(in_names)
    all_in_names = list(in_names) + list(out_names)
    if partition_name is not None:
        all_in_names.append(partition_name)

    def _body(*args):
        operands = list(args)
        if partition_name is not None:
            operands.append(bass2jax.partition_id_tensor())
        outs = bass2jax._bass_exec_p.bind(
            *operands,
            out_avals=tuple(out_avals),
            in_names=tuple(all_in_names),
            out_names=tuple(out_names),
            lowering_input_output_aliases=(),
            sim_require_finite=True,
            sim_require_nnan=True,
            nc=nc,
        )
        return tuple(outs)

    devices = jax.devices()[:n_cores]
    mesh = bass2jax.Mesh(np.asarray(devices), ("core",))
    in_specs = (bass2jax.PartitionSpec("core"),) * (n_params + len(out_names))
    out_specs = (bass2jax.PartitionSpec("core"),) * len(out_names)
    fn = jax.jit(
        bass2jax.shard_map(
            _body, mesh=mesh, in_specs=in_specs, out_specs=out_specs, check_rep=False
        ),
        keep_unused=True,
    )
    per_core = [[np.asarray(m[name]) for name in in_names] for m in in_maps]
    concat_in = [
        np.concatenate([per_core[c][i] for c in range(n_cores)], axis=0)
        for i in range(n_params)
    ]
    concat_zeros = [
        np.zeros((n_cores * z.shape[0], *z.shape[1:]), z.dtype) for z in zero_outs
    ]
    sh = NamedSharding(mesh, bass2jax.PartitionSpec("core"))
    dev_args = [jax.device_put(a, sh) for a in concat_in + concat_zeros]
    return fn, dev_args


def bench(x, h_cur, c_cur, weight, bias, dt_mm=None, ks=(4, 16)):
    """Returns estimated per-call device exec time in ns (pipelined slope)."""
    import time as _time

    import jax

    if dt_mm is None:
        dt_mm = mybir.dt.float8e4
    nc = _build(dt_mm)
    in_maps = _prep_inputs(
        np.asarray(x), np.asarray(h_cur), np.asarray(c_cur), np.asarray(weight), np.asarray(bias), dt_mm
    )
    fn, dev_args = _make_timing_fn(nc, in_maps)
    for _ in range(2):
        outs = fn(*dev_args)
        jax.block_until_ready(outs)

    def timed(k):
        t0 = _time.perf_counter()
        outs = None
        for _ in range(k):
            outs = fn(*dev_args)
        jax.block_until_ready(outs)
        return _time.perf_counter() - t0

    times = {}
    for k in ks:
        times[k] = min(timed(k) for _ in range(3))
    k_lo, k_hi = min(ks), max(ks)
    slope = (times[k_hi] - times[k_lo]) / (k_hi - k_lo)
    return slope * 1e9, times



# revision 6
# speedup vs baseline: 1.0911x; 1.0911x over previous
"""ConvLSTM2D cell on 8 Trainium2 NeuronCores.

Data-parallel over batch: B=16 -> 2 images per core.

fp8 path (default): the 3x3 conv over concat([x, h]) is computed with
fp8e4m3 DoubleRow matmuls (2 K=128 tiles contracted per instruction at
0.5 cycles/output-column -- 2x the fp16 rate). Precision is recovered
with a 3-term residual expansion: w*a ~= w8*a8 + w8*ar + wr*a8, where
w8/a8 are fp8 quantizations (weights pre-scaled x32 into fp8 normal
range) and wr/ar are fp8 quantizations of the leftover residuals stored
at the SAME scale (they land in fp8's subnormal range, which is exactly
fine: the absolute subnormal step 2^-9 bounds the residual error).
All three terms share one PSUM accumulator; the x32 weight scale is
undone by the activation's scale parameter. Measured end-to-end error
~5e-3 vs the f32 reference (vs ~1e-3 for the fp16 path).

Per (gate, 512-pixel chunk): 21 DoubleRow matmuls (7 per term) instead
of 14 fp16 matmuls. Each DR matmul contracts two conv taps' K-tiles
selected by a strided access pattern over one padded SBUF image tile
img[128ch, 3 planes, 66, 66] (plane 0 = padded h; plane 1 = x packed
with x-shifted-left in channels 64-127; plane 2 = x packed with
x-shifted-up), so every tap pair is a pure AP offset. ScalarE applies
scale+bias+sigmoid/tanh straight out of PSUM; VectorE does the LSTM
elementwise math.

fp16 path kept for fallback/AB (dt_mm=float16): 14 matmuls per (gate,
chunk), see git history for details.
"""

import sys

if "/opt/trn_rl_repo" not in sys.path:
    sys.path.insert(0, "/opt/trn_rl_repo")

import numpy as np

import concourse.bass as bass
import concourse.tile as tile
from bass_rust import AP
from concourse import bacc, mybir
from concourse.bass_utils import run_bass_kernel_spmd

N_CORES = 8
B, C_IN, C_HID, H, W = 16, 64, 128, 64, 64
B_LOC = B // N_CORES  # 2 images per core
HP = H + 2  # padded
WP = W + 2
PL = HP * WP  # plane size in elements
ROWS_PER_CHUNK = 8  # 8 rows x 64 cols = 512 pixels per matmul chunk
NCH = H // ROWS_PER_CHUNK  # chunks per image
TAPS = [(kh, kw) for kh in range(3) for kw in range(3)]
W_SCALE = 32.0  # fp8 weight pre-scale (undone in the activation)

# fp8 image layout: 5 pre-shifted 64-wide planes per image, each 66 rows
# (1 zero guard row top+bottom). Horizontal tap shifts are baked into the
# planes so every 8x64 window is a single CONTIGUOUS 512-element block --
# DoubleRow rhs APs stay 3-dim [128, 2, 512] (4-dim strided DR ifmaps
# crash the device). Planes: 0..2 = h shifted dw=-1/0/+1; 3 = xp pack
# [x(dw=-1) | x(dw=0)] in channel halves; 4 = xq pack [x(dw=+1) |
# x(dw=+1) shifted up one row].
N_PLANES = 5
PS = (H + 2) * W  # plane size: 66 rows x 64 cols

# DoubleRow pair table: (plane of ktile0, dv of ktile0, ktile delta).
# dv is the vertical tap shift (kh-1); ktile1 sits at +delta.
_DR_PAIRS = [
    (0, -1, PS),  # h(0,0) + h(0,1)
    (0, 0, PS),  # h(1,0) + h(1,1)
    (0, 1, PS),  # h(2,0) + h(2,1)
    (2, -1, W),  # h(0,2) + h(1,2)
    (3, -1, W),  # x(0,0)+x(0,1) + x(1,0)+x(1,1)
    (3, 1, PS - 2 * W),  # x(2,0)+x(2,1) + x(0,2)+x(1,2)
    (2, 1, 2 * PS),  # h(2,2) + x(2,2) (ktile1 ch64-127 weights are 0)
]

_cache = {}


def _dedupe_ldweights(nc):
    """Remove InstLdweights whose stationary AP repeats the previous PE
    weight load in the same block (and which carry no sync). The PE array
    keeps the loaded weights across matmuls, so consecutive matmuls sharing
    a stationary only need the first load; the emission layer inserts one
    per matmul unconditionally. Verified bit-identical on HW."""
    n_del = 0
    for blk in nc.m.functions[0].blocks:
        cur = None
        dels = []
        for inst in blk.instructions:
            if isinstance(inst, mybir.InstLdweights):
                key = (str(inst.ins[0]), str(inst.perf_mode), str(inst.is_transpose),
                       str(inst.tile_position))
                si = inst.sync_info
                sync_free = si is None or (len(si.on_wait) == 0 and len(si.on_update) == 0)
                if key == cur and sync_free:
                    dels.append(inst)
                    n_del += 1
                else:
                    cur = key
            elif isinstance(inst, mybir.InstMatmult):
                if inst.is_transpose:
                    cur = None
        for inst in dels:
            blk.instructions.remove(inst)
    return n_del


def _dr_view(t, p, h0):
    """rhs AP [128, 2, 512] for DoubleRow pair p at chunk row h0 on an
    image tile t of shape [128, N_PLANES, 66, 64]."""
    plane, dv, delta = _DR_PAIRS[p]
    full = t[:]
    off = full.offset + plane * PS + (h0 + dv + 1) * W
    return AP(full.tensor, off, [full.ap[0], (delta, 2), (1, ROWS_PER_CHUNK * W)])


def _build_fp8(trace=False, unroll=1):
    f32 = mybir.dt.float32
    f8 = mybir.dt.float8e4
    nc = bacc.Bacc("TRN2", target_bir_lowering=False, debug=False, num_devices=N_CORES)

    img8_ap = nc.dram_tensor("img8", [B_LOC, 128, N_PLANES, H + 2, W], f8, kind="ExternalInput").ap()
    imgr8_ap = nc.dram_tensor("imgr8", [B_LOC, 128, N_PLANES, H + 2, W], f8, kind="ExternalInput").ap()
    c_ap = nc.dram_tensor("c", [B_LOC, C_HID, H * W], f32, kind="ExternalInput").ap()
    # wdr: [K=128, set(0=w8,1=wr), gate, pair, ktile, M=128]
    wdr_ap = nc.dram_tensor("wdr", [128, 2, 4, 7, 2, 128], f8, kind="ExternalInput").ap()
    bias_ap = nc.dram_tensor("biasT", [C_HID, 4], f32, kind="ExternalInput").ap()
    hn_ap = nc.dram_tensor("hn", [B_LOC, C_HID, H * W], f32, kind="ExternalOutput").ap()
    cn_ap = nc.dram_tensor("cn", [B_LOC, C_HID, H * W], f32, kind="ExternalOutput").ap()

    with tile.TileContext(nc) as tc:
        with (
            tc.tile_pool(name="weights", bufs=1) as wpool,
            tc.tile_pool(name="imgs", bufs=2) as ipool,
            tc.tile_pool(name="cstate", bufs=2) as cpool,
            tc.tile_pool(name="psum", bufs=1, space="PSUM") as ppool,
            tc.tile_pool(name="acts", bufs=2) as apool,
            tc.tile_pool(name="outs", bufs=2) as opool,
        ):
            w_t = wpool.tile([128, 2, 4, 7, 2, 128], f8, tag="wdr")
            bias_t = wpool.tile([C_HID, 4], f32, tag="bias")
            nc.sync.dma_start(w_t[:], wdr_ap[:])
            nc.sync.dma_start(bias_t[:], bias_ap[:])

            i8 = []
            ir8 = []
            for b in range(B_LOC):
                i8_b = ipool.tile([128, N_PLANES, H + 2, W], f8, tag="img8")
                ir8_b = ipool.tile([128, N_PLANES, H + 2, W], f8, tag="imgr8")
                nc.sync.dma_start(i8_b[:], img8_ap[b])
                nc.sync.dma_start(ir8_b[:], imgr8_ap[b])
                i8.append(i8_b)
                ir8.append(ir8_b)

            # Blocked for stationary-weight reuse: (2 gates x 4 chunks) = 8
            # PSUM banks accumulate concurrently while each w8/wr stationary
            # is loaded once and streamed over 8/4 matmuls; the redundant
            # per-matmul LdWeights are deduped after TileContext exit.
            dr = mybir.MatmulPerfMode.DoubleRow
            NCS = B * NCH // N_CORES // 4  # chunk sets of 4 per core
            for _rep in range(unroll):
                for cs in range(NCS):
                    chunks = [4 * cs + j for j in range(4)]  # global chunk ids
                    c_sl = {}
                    for j, gc in enumerate(chunks):
                        b, ch = divmod(gc, NCH)
                        h0 = ch * ROWS_PER_CHUNK
                        t = cpool.tile([C_HID, 512], f32, tag=f"c{j}", name=f"c_{_rep}_{gc}")
                        nc.sync.dma_start(t[:], c_ap[b][:, h0 * W : (h0 + ROWS_PER_CHUNK) * W])
                        c_sl[gc] = t
                    acts = {}
                    for gp in range(2):
                        gates = (2 * gp, 2 * gp + 1)
                        accs = {}
                        for gi, g in enumerate(gates):
                            for j, gc in enumerate(chunks):
                                accs[(g, gc)] = ppool.tile(
                                    [C_HID, 512], f32, tag=f"acc{gi}{j}",
                                    name=f"acc_{_rep}_{g}_{gc}",
                                )
                        for p in range(7):
                            for g in gates:
                                for gc in chunks:
                                    b, ch = divmod(gc, NCH)
                                    h0 = ch * ROWS_PER_CHUNK
                                    v8 = _dr_view(i8[b], p, h0)
                                    vr = _dr_view(ir8[b], p, h0)
                                    nc.tensor.matmul(
                                        accs[(g, gc)][:], w_t[:, 0, g, p], v8,
                                        start=(p == 0), stop=False, perf_mode=dr,
                                    )
                                    nc.tensor.matmul(
                                        accs[(g, gc)][:], w_t[:, 0, g, p], vr,
                                        start=False, stop=False, perf_mode=dr,
                                    )
                                for gc in chunks:
                                    b, ch = divmod(gc, NCH)
                                    h0 = ch * ROWS_PER_CHUNK
                                    v8 = _dr_view(i8[b], p, h0)
                                    nc.tensor.matmul(
                                        accs[(g, gc)][:], w_t[:, 1, g, p], v8,
                                        start=False, stop=(p == 6), perf_mode=dr,
                                    )
                        for gi, g in enumerate(gates):
                            func = (
                                mybir.ActivationFunctionType.Tanh
                                if g == 3
                                else mybir.ActivationFunctionType.Sigmoid
                            )
                            for j, gc in enumerate(chunks):
                                act_t = apool.tile(
                                    [C_HID, 512], f32, tag=f"act{g}{j}",
                                    name=f"act_{_rep}_{g}_{gc}",
                                )
                                nc.scalar.activation(
                                    act_t[:], accs[(g, gc)][:], func,
                                    bias=bias_t[:, g : g + 1], scale=1.0 / W_SCALE,
                                )
                                acts[(g, gc)] = act_t

                    for gc in chunks:
                        b, ch = divmod(gc, NCH)
                        h0 = ch * ROWS_PER_CHUNK
                        i_t, f_t, o_t, g_t = (acts[(g, gc)] for g in range(4))
                        ig = opool.tile([C_HID, 512], f32, tag="ig")
                        nc.vector.tensor_mul(ig[:], i_t[:], g_t[:])
                        fc = opool.tile([C_HID, 512], f32, tag="fc")
                        nc.vector.tensor_mul(fc[:], f_t[:], c_sl[gc][:])
                        cn_t = opool.tile([C_HID, 512], f32, tag="cn")
                        nc.vector.tensor_add(cn_t[:], fc[:], ig[:])
                        nc.sync.dma_start(cn_ap[b][:, h0 * W : (h0 + ROWS_PER_CHUNK) * W], cn_t[:])
                        th_t = opool.tile([C_HID, 512], f32, tag="th")
                        nc.scalar.activation(th_t[:], cn_t[:], mybir.ActivationFunctionType.Tanh)
                        hn_t = opool.tile([C_HID, 512], f32, tag="hn")
                        nc.vector.tensor_mul(hn_t[:], o_t[:], th_t[:])
                        nc.sync.dma_start(hn_ap[b][:, h0 * W : (h0 + ROWS_PER_CHUNK) * W], hn_t[:])

    n_del = _dedupe_ldweights(nc)
    assert n_del > 0, "expected redundant ldweights to be removed"
    nc.compile()
    return nc


def _build_fp16(dt_mm, trace=False, unroll=1):
    f32 = mybir.dt.float32
    nc = bacc.Bacc("TRN2", target_bir_lowering=False, debug=False, num_devices=N_CORES)

    x_ap = nc.dram_tensor("x", [B_LOC, 2 * C_IN, HP, WP], dt_mm, kind="ExternalInput").ap()
    x2_ap = nc.dram_tensor("x2", [B_LOC, 2 * C_IN, HP, WP], dt_mm, kind="ExternalInput").ap()
    h_ap = nc.dram_tensor("h", [B_LOC, C_HID, HP, WP], dt_mm, kind="ExternalInput").ap()
    c_ap = nc.dram_tensor("c", [B_LOC, C_HID, H * W], f32, kind="ExternalInput").ap()
    wxp_ap = nc.dram_tensor("wxp", [3, 2 * C_IN, 4 * C_HID], dt_mm, kind="ExternalInput").ap()
    wxr_ap = nc.dram_tensor("wxr", [2 * C_IN, 4 * C_HID], dt_mm, kind="ExternalInput").ap()
    wx3_ap = nc.dram_tensor("wx3", [C_IN, 4 * C_HID], dt_mm, kind="ExternalInput").ap()
    wh_ap = nc.dram_tensor("wh", [9, C_HID, 4 * C_HID], dt_mm, kind="ExternalInput").ap()
    bias_ap = nc.dram_tensor("biasT", [C_HID, 4], f32, kind="ExternalInput").ap()
    hn_ap = nc.dram_tensor("hn", [B_LOC, C_HID, H * W], f32, kind="ExternalOutput").ap()
    cn_ap = nc.dram_tensor("cn", [B_LOC, C_HID, H * W], f32, kind="ExternalOutput").ap()

    with tile.TileContext(nc) as tc:
        with (
            tc.tile_pool(name="weights", bufs=1) as wpool,
            tc.tile_pool(name="imgs", bufs=2) as ipool,
            tc.tile_pool(name="cstate", bufs=3) as cpool,
            tc.tile_pool(name="psum", bufs=8, space="PSUM") as ppool,
            tc.tile_pool(name="acts", bufs=2) as apool,
            tc.tile_pool(name="outs", bufs=3) as opool,
        ):
            wh_t = wpool.tile([C_HID, 9, 4 * C_HID], dt_mm, tag="wh")
            wxp_t = wpool.tile([2 * C_IN, 3, 4 * C_HID], dt_mm, tag="wxp")
            wxr_t = wpool.tile([2 * C_IN, 4 * C_HID], dt_mm, tag="wxr")
            wx3_t = wpool.tile([C_IN, 4 * C_HID], dt_mm, tag="wx3")
            bias_t = wpool.tile([C_HID, 4], f32, tag="bias")
            nc.sync.dma_start(wh_t[:], wh_ap.rearrange("t k m -> k t m"))
            nc.sync.dma_start(wxp_t[:], wxp_ap.rearrange("t k m -> k t m"))
            nc.sync.dma_start(wxr_t[:], wxr_ap[:])
            nc.sync.dma_start(wx3_t[:], wx3_ap[:])
            nc.sync.dma_start(bias_t[:], bias_ap[:])

            hp = []
            xp = []
            xq = []
            for b in range(B_LOC):
                hp_b = ipool.tile([C_HID, HP, WP], dt_mm, tag="hp")
                xp_b = ipool.tile([2 * C_IN, HP, WP], dt_mm, tag="xp")
                xq_b = ipool.tile([2 * C_IN, HP, WP], dt_mm, tag="xq")
                nc.sync.dma_start(hp_b[:], h_ap[b])
                nc.sync.dma_start(xp_b[:], x_ap[b])
                nc.sync.dma_start(xq_b[:], x2_ap[b])
                hp.append(hp_b)
                xp.append(xp_b)
                xq.append(xq_b)

            for _rep in range(unroll):
                for b in range(B_LOC):
                    for ch in range(NCH):
                        h0 = ch * ROWS_PER_CHUNK
                        c_sl = cpool.tile([C_HID, 512], f32, tag="c", name=f"c_{_rep}_{b}_{ch}")
                        nc.sync.dma_start(c_sl[:], c_ap[b][:, h0 * W : (h0 + ROWS_PER_CHUNK) * W])
                        gate_sb = []
                        for g in range(4):
                            acc = ppool.tile([C_HID, 512], f32, tag="acc")
                            n_mm = 14
                            mm = 0
                            for kh, kw in TAPS:
                                nc.tensor.matmul(
                                    acc[:],
                                    wh_t[:, kh * 3 + kw, g * C_HID : (g + 1) * C_HID],
                                    hp[b][:, h0 + kh : h0 + kh + ROWS_PER_CHUNK, kw : kw + W],
                                    start=(mm == 0),
                                    stop=(mm == n_mm - 1),
                                )
                                mm += 1
                            for kh in range(3):
                                nc.tensor.matmul(
                                    acc[:],
                                    wxp_t[:, kh, g * C_HID : (g + 1) * C_HID],
                                    xp[b][:, h0 + kh : h0 + kh + ROWS_PER_CHUNK, 0:W],
                                    start=(mm == 0),
                                    stop=(mm == n_mm - 1),
                                )
                                mm += 1
                            nc.tensor.matmul(
                                acc[:],
                                wxr_t[:, g * C_HID : (g + 1) * C_HID],
                                xq[b][:, h0 : h0 + ROWS_PER_CHUNK, 2 : 2 + W],
                                start=(mm == 0),
                                stop=(mm == n_mm - 1),
                            )
                            mm += 1
                            nc.tensor.matmul(
                                acc[:],
                                wx3_t[:, g * C_HID : (g + 1) * C_HID],
                                xp[b][0:C_IN, h0 + 2 : h0 + 2 + ROWS_PER_CHUNK, 2 : 2 + W],
                                start=(mm == 0),
                                stop=(mm == n_mm - 1),
                            )
                            mm += 1
                            act_t = apool.tile([C_HID, 512], f32, tag=f"gate{g}")
                            func = (
                                mybir.ActivationFunctionType.Tanh
                                if g == 3
                                else mybir.ActivationFunctionType.Sigmoid
                            )
                            nc.scalar.activation(act_t[:], acc[:], func, bias=bias_t[:, g : g + 1])
                            gate_sb.append(act_t)

                        i_t, f_t, o_t, g_t = gate_sb
                        ig = opool.tile([C_HID, 512], f32, tag="ig")
                        nc.vector.tensor_mul(ig[:], i_t[:], g_t[:])
                        fc = opool.tile([C_HID, 512], f32, tag="fc")
                        nc.vector.tensor_mul(fc[:], f_t[:], c_sl[:])
                        cn_t = opool.tile([C_HID, 512], f32, tag="cn")
                        nc.vector.tensor_add(cn_t[:], fc[:], ig[:])
                        nc.sync.dma_start(cn_ap[b][:, h0 * W : (h0 + ROWS_PER_CHUNK) * W], cn_t[:])
                        th_t = opool.tile([C_HID, 512], f32, tag="th")
                        nc.scalar.activation(th_t[:], cn_t[:], mybir.ActivationFunctionType.Tanh)
                        hn_t = opool.tile([C_HID, 512], f32, tag="hn")
                        nc.vector.tensor_mul(hn_t[:], o_t[:], th_t[:])
                        nc.sync.dma_start(hn_ap[b][:, h0 * W : (h0 + ROWS_PER_CHUNK) * W], hn_t[:])

    nc.compile()
    return nc


def _build(dt_mm=mybir.dt.float8e4, trace=False, unroll=1):
    key = (dt_mm, trace, unroll)
    if key in _cache:
        return _cache[key]
    if dt_mm == mybir.dt.float8e4:
        nc = _build_fp8(trace, unroll)
    else:
        nc = _build_fp16(dt_mm, trace, unroll)
    _cache[key] = nc
    return nc


def _pad_images(x, h_cur):
    """Returns [B, 128, 3, HP, WP] f32: plane 0 = padded h, plane 1 = xp
    (x | x-left), plane 2 = xq (x | x-up)."""
    hpad = np.zeros((B, C_HID, HP, WP), dtype=np.float32)
    hpad[:, :, 1 : H + 1, 1 : W + 1] = h_cur
    xpad = np.zeros((B, 2 * C_IN, HP, WP), dtype=np.float32)
    xpad[:, :C_IN, 1 : H + 1, 1 : W + 1] = x
    xpad[:, C_IN:, :, : WP - 1] = xpad[:, :C_IN, :, 1:]
    xqpad = np.zeros((B, 2 * C_IN, HP, WP), dtype=np.float32)
    xqpad[:, :C_IN] = xpad[:, :C_IN]
    xqpad[:, C_IN:, : HP - 1, :] = xpad[:, :C_IN, 1:, :]
    return np.stack([hpad, xpad, xqpad], axis=2)


def _shift_w(a, dw):
    """Horizontal shift with zero fill: out[..., c] = a[..., c + dw]."""
    out = np.zeros_like(a)
    if dw == 0:
        out[:] = a
    elif dw == 1:
        out[..., : W - 1] = a[..., 1:]
    else:
        out[..., 1:] = a[..., : W - 1]
    return out


def _fp8_planes(x, h_cur):
    """[B, 128, N_PLANES, 66, 64] f32. Planes 0-2: h at dw=-1/0/+1 with one
    zero guard row top+bottom. Plane 3: [x(dw=-1) | x(dw=0)] channel halves.
    Plane 4: [x(dw=+1) | x(dw=+1) up one row] (ch 64-127 laid out so the
    dv=-1 window reads x(dw=+1) at dv=0)."""
    planes = np.zeros((B, 128, N_PLANES, H + 2, W), dtype=np.float32)
    for i, dw in enumerate([-1, 0, 1]):
        planes[:, :, i, 1 : H + 1, :] = _shift_w(h_cur, dw)
    xp1 = _shift_w(x, 1)
    planes[:, :C_IN, 3, 1 : H + 1, :] = _shift_w(x, -1)
    planes[:, C_IN:, 3, 1 : H + 1, :] = x
    planes[:, :C_IN, 4, 1 : H + 1, :] = xp1
    planes[:, C_IN:, 4, 0:H, :] = xp1
    return planes


def _prep_inputs_fp8(x, h_cur, c_cur, weight, bias):
    np8 = mybir.dt.np(mybir.dt.float8e4)
    img = _fp8_planes(x, h_cur)  # [B, 128, N_PLANES, 66, 64] f32
    img8 = img.astype(np8)
    imgr8 = (img - img8.astype(np.float32)).astype(np8)

    # weights: [512, 192, 3, 3] -> [3, 3, 192, 512], pre-scale x32, split
    wt = np.ascontiguousarray(weight.transpose(2, 3, 1, 0)).astype(np.float32) * W_SCALE
    w8 = wt.astype(np8)
    wr = (wt - w8.astype(np.float32)).astype(np8)

    wdr = np.zeros((128, 2, 4, 7, 2, 128), dtype=np8)
    for s, S in enumerate([w8, wr]):
        Sf = S  # fp8 array; slicing fine
        Sh = Sf[:, :, C_IN:, :]  # [3,3,128,512]
        Sx = Sf[:, :, :C_IN, :]  # [3,3,64,512]
        for g in range(4):
            msl = slice(g * C_HID, (g + 1) * C_HID)
            pairs = [
                (Sh[0, 0, :, msl], Sh[0, 1, :, msl]),
                (Sh[1, 0, :, msl], Sh[1, 1, :, msl]),
                (Sh[2, 0, :, msl], Sh[2, 1, :, msl]),
                (Sh[0, 2, :, msl], Sh[1, 2, :, msl]),
                (
                    np.concatenate([Sx[0, 0, :, msl], Sx[0, 1, :, msl]], axis=0),
                    np.concatenate([Sx[1, 0, :, msl], Sx[1, 1, :, msl]], axis=0),
                ),
                (
                    np.concatenate([Sx[2, 0, :, msl], Sx[2, 1, :, msl]], axis=0),
                    np.concatenate([Sx[0, 2, :, msl], Sx[1, 2, :, msl]], axis=0),
                ),
                (
                    Sh[2, 2, :, msl],
                    np.concatenate(
                        [Sx[2, 2, :, msl], np.zeros((C_IN, C_HID), dtype=np8)], axis=0
                    ),
                ),
            ]
            for p, (kt0, kt1) in enumerate(pairs):
                wdr[:, s, g, p, 0, :] = kt0
                wdr[:, s, g, p, 1, :] = kt1

    biasT = np.ascontiguousarray(bias.reshape(4, C_HID).T, dtype=np.float32)
    c3 = np.ascontiguousarray(c_cur.reshape(B, C_HID, H * W), dtype=np.float32)

    in_maps = []
    for i in range(N_CORES):
        s = slice(i * B_LOC, (i + 1) * B_LOC)
        in_maps.append(
            {
                "img8": np.ascontiguousarray(img8[s]),
                "imgr8": np.ascontiguousarray(imgr8[s]),
                "c": c3[s],
                "wdr": wdr,
                "biasT": biasT,
            }
        )
    return in_maps


def _prep_inputs_fp16(x, h_cur, c_cur, weight, bias, dt_mm):
    if dt_mm == mybir.dt.bfloat16:
        import ml_dtypes

        cast = lambda a: np.asarray(a, dtype=ml_dtypes.bfloat16)
    elif dt_mm == mybir.dt.float16:
        cast = lambda a: np.asarray(a, dtype=np.float16)
    else:
        cast = lambda a: np.ascontiguousarray(a, dtype=np.float32)

    wt = np.ascontiguousarray(weight.transpose(2, 3, 1, 0))  # [3,3,ci,co]
    wx = wt[:, :, :C_IN, :]
    wh = cast(wt[:, :, C_IN:, :].reshape(9, C_HID, 4 * C_HID))
    wxp = cast(np.concatenate([wx[:, 0, :, :], wx[:, 1, :, :]], axis=1))
    wxr = cast(np.concatenate([wx[0, 2, :, :], wx[1, 2, :, :]], axis=0))
    wx3 = cast(np.ascontiguousarray(wx[2, 2, :, :]))
    biasT = np.ascontiguousarray(bias.reshape(4, C_HID).T, dtype=np.float32)
    c3 = np.ascontiguousarray(c_cur.reshape(B, C_HID, H * W), dtype=np.float32)

    img = _pad_images(x, h_cur)  # [B, 128, 3, HP, WP]
    hpad = img[:, :, 0]
    xpad = img[:, :, 1]
    x2pad = img[:, :, 2]

    in_maps = []
    for i in range(N_CORES):
        s = slice(i * B_LOC, (i + 1) * B_LOC)
        in_maps.append(
            {
                "x": cast(xpad[s]),
                "x2": cast(x2pad[s]),
                "h": cast(hpad[s]),
                "c": c3[s],
                "wxp": wxp,
                "wxr": wxr,
                "wx3": wx3,
                "wh": wh,
                "biasT": biasT,
            }
        )
    return in_maps


def _prep_inputs(x, h_cur, c_cur, weight, bias, dt_mm=mybir.dt.float8e4):
    if dt_mm == mybir.dt.float8e4:
        return _prep_inputs_fp8(x, h_cur, c_cur, weight, bias)
    return _prep_inputs_fp16(x, h_cur, c_cur, weight, bias, dt_mm)


def run(x, h_cur, c_cur, weight, bias, dt_mm=mybir.dt.float8e4, trace=False):
    x = np.asarray(x)
    h_cur = np.asarray(h_cur)
    c_cur = np.asarray(c_cur)
    weight = np.asarray(weight)
    bias = np.asarray(bias)
    nc = _build(dt_mm, trace)
    in_maps = _prep_inputs(x, h_cur, c_cur, weight, bias, dt_mm)
    res = run_bass_kernel_spmd(nc, in_maps, list(range(N_CORES)), trace=trace)
    hn = np.concatenate([res.results[i]["hn"] for i in range(N_CORES)], axis=0)
    cn = np.concatenate([res.results[i]["cn"] for i in range(N_CORES)], axis=0)
    hn = hn.reshape(B, C_HID, H, W).astype(np.float32)
    cn = cn.reshape(B, C_HID, H, W).astype(np.float32)
    return (hn, cn), res


def kernel(x, h_cur, c_cur, weight, bias):
    (hn, cn), _ = run(x, h_cur, c_cur, weight, bias)
    return hn, cn


def _make_timing_fn(nc, in_maps):
    """Non-donating jitted runner with device-resident inputs, for
    throughput timing (slope of wall time vs iteration count)."""
    import jax
    from jax.sharding import NamedSharding

    from concourse import bass2jax, mybir as _mybir

    bass2jax.install_neuronx_cc_hook()
    n_cores = len(in_maps)
    partition_name = nc.partition_id_tensor.name if nc.partition_id_tensor else None
    in_names, out_names, out_avals, zero_outs = [], [], [], []
    for alloc in nc.m.functions[0].allocations:
        if not isinstance(alloc, _mybir.MemoryLocationSet):
            continue
        name = alloc.memorylocations[0].name
        if alloc.kind == "ExternalInput":
            if name != partition_name:
                in_names.append(name)
        elif alloc.kind == "ExternalOutput":
            out_names.append(name)
            shape = tuple(alloc.tensor_shape)
            dtype = _mybir.dt.np(alloc.dtype)
            out_avals.append(jax.core.ShapedArray(shape, dtype))
            zero_outs.append(np.zeros(shape, dtype))
    n_params = len(in_names)
    all_in_names = list(in_names) + list(out_names)
    if partition_name is not None:
        all_in_names.append(partition_name)

    def _body(*args):
        operands = list(args)
        if partition_name is not None:
            operands.append(bass2jax.partition_id_tensor())
        outs = bass2jax._bass_exec_p.bind(
            *operands,
            out_avals=tuple(out_avals),
            in_names=tuple(all_in_names),
            out_names=tuple(out_names),
            lowering_input_output_aliases=(),
            sim_require_finite=True,
            sim_require_nnan=True,
            nc=nc,
        )
        return tuple(outs)

    devices = jax.devices()[:n_cores]
    mesh = bass2jax.Mesh(np.asarray(devices), ("core",))
    in_specs = (bass2jax.PartitionSpec("core"),) * (n_params + len(out_names))
    out_specs = (bass2jax.PartitionSpec("core"),) * len(out_names)
    fn = jax.jit(
        bass2jax.shard_map(
            _body, mesh=mesh, in_specs=in_specs, out_specs=out_specs, check_rep=False
        ),
        keep_unused=True,
    )
    per_core = [[np.asarray(m[name]) for name in in_names] for m in in_maps]
    concat_in = [
        np.concatenate([per_core[c][i] for c in range(n_cores)], axis=0)
        for i in range(n_params)
    ]
    concat_zeros = [
        np.zeros((n_cores * z.shape[0], *z.shape[1:]), z.dtype) for z in zero_outs
    ]
    sh = NamedSharding(mesh, bass2jax.PartitionSpec("core"))
    dev_args = [jax.device_put(a, sh) for a in concat_in + concat_zeros]
    return fn, dev_args


def bench(x, h_cur, c_cur, weight, bias, dt_mm=None, ks=(4, 16)):
    """Returns estimated per-call device exec time in ns (pipelined slope)."""
    import time as _time

    import jax

    if dt_mm is None:
        dt_mm = mybir.dt.float8e4
    nc = _build(dt_mm)
    in_maps = _prep_inputs(
        np.asarray(x), np.asarray(h_cur), np.asarray(c_cur), np.asarray(weight), np.asarray(bias), dt_mm
    )
    fn, dev_args = _make_timing_fn(nc, in_maps)
    for _ in range(2):
        outs = fn(*dev_args)
        jax.block_until_ready(outs)

    def timed(k):
        t0 = _time.perf_counter()
        outs = None
        for _ in range(k):
            outs = fn(*dev_args)
        jax.block_until_ready(outs)
        return _time.perf_counter() - t0

    times = {}
    for k in ks:
        times[k] = min(timed(k) for _ in range(3))
    k_lo, k_hi = min(ks), max(ks)
    slope = (times[k_hi] - times[k_lo]) / (k_hi - k_lo)
    return slope * 1e9, times

